# revision 23
# baseline (speedup 1.0000x reference)
"""Trainium2 Bass kernel for nn_BoundleAdjustment (2M observations).

Two launches on all 8 NeuronCores (observations data-parallel, M/8 per core):

Launch A (device): converts the 4096-row pose table (translation+quaternion)
into per-pose rotation matrices R = f(q/|q|) on the Vector engine
([128, 32] planar layout, one reciprocal for the 2/|q|^2 scale).

Host staging (indexing/layout only): gathers the derived R table, raw pose
translations, and patch rows by poses_idx/patch_idx, casts the per-
observation record planes to fp16, and lays them out as two contiguous
blocks per chunk so each chunk needs only two big DMAs.

Launch B (device): streams fp16 planes through SBUF in 2 chunks.
Rotation + residual math runs in fp16 on the Vector engine (2x DVE mode);
squares/sqrts/arctans on the Scalar engine; the azimuth uses the
half-angle identity az = 2*atan(ry/(rho+rx)) which needs no quadrant
fixup; the two reciprocals run in f32 via reciprocal_approx_fast with
max(x,1e-30) guards so no inf/NaN can form.
"""

import numpy as np

M = 2097152
NCORES = 8
N = M // NCORES
P = 128
COLS = N // P            # 2048
CC = 1024                # chunk cols
NCH = COLS // CC         # 2 chunks
NPOSE = 4096
PC = NPOSE // P          # 32 cols for pose table

# input groups (fp16), DMA'd per chunk in need-order so compute starts
# as soon as the rotation operands have landed:
#   tR:  R00 R01 R02 R10 R11 R12 R20 R21 R22   (9 planes, first)
#   tP:  px py pz                              (3 planes)
#   tTW: tx ty tz W                            (4 planes)
#   tXY: X Y Z                                 (3 planes, last)
NPA = 7                  # legacy (unused)
NPB_ = 12                # legacy (unused)

_CACHE = {}


# launch A staged layout: 22 blocks of 32 cols, products prod_k = QA_k * QB_k
#   0-5   PL1 = yy xx xx xy xz yz      6-11  PL2 = zz zz yy wz wy wx
#   12-14 MN1 = xy xz yz               15-17 MN2 = wz wy wx
#   18-21 SS  = xx yy zz ww
# plus  = PL1+PL2 = [d00 d11 d22 o10 o02 o21], minus = MN1-MN2 = [o01 o20 o12]
_QA_IDX = [1, 0, 0, 0, 0, 1,  2, 2, 1, 3, 3, 3,  0, 0, 1,  3, 3, 3,  0, 1, 2, 3]
_QB_IDX = [1, 0, 0, 1, 2, 2,  2, 2, 1, 2, 1, 0,  1, 2, 2,  2, 1, 0,  0, 1, 2, 3]
NQB = 22


def _build_posetab():
    from concourse import bacc, mybir

    nc = bacc.Bacc("TRN2", target_bir_lowering=False, debug=False,
                   num_devices=NCORES)
    f32 = mybir.dt.float32
    OP = mybir.AluOpType
    qa_d = nc.declare_dram_parameter("qa", [P, NQB * PC], f32, isOutput=False)
    qb_d = nc.declare_dram_parameter("qb", [P, NQB * PC], f32, isOutput=False)
    r_d = nc.declare_dram_parameter("rtab", [P, 9 * PC], f32, isOutput=True)

    qa_s = nc.alloc_sbuf_tensor("qa_s", [P, NQB * PC], f32)
    qb_s = nc.alloc_sbuf_tensor("qb_s", [P, NQB * PC], f32)
    prod = nc.alloc_sbuf_tensor("prod", [P, NQB * PC], f32)
    plus = nc.alloc_sbuf_tensor("plus", [P, 6 * PC], f32)
    minus = nc.alloc_sbuf_tensor("minus", [P, 3 * PC], f32)
    s2 = nc.alloc_sbuf_tensor("s2", [P, 2 * PC], f32)
    d1 = nc.alloc_sbuf_tensor("d1", [P, PC], f32)
    dh = nc.alloc_sbuf_tensor("dh", [P, PC], f32)
    u = nc.alloc_sbuf_tensor("u", [P, PC], f32)
    dgm = nc.alloc_sbuf_tensor("dgm", [P, 3 * PC], f32)
    rt = nc.alloc_sbuf_tensor("rt", [P, 9 * PC], f32)

    in_sem = nc.alloc_semaphore("in_sem")
    done_sem = nc.alloc_semaphore("done_sem")
    out_sem = nc.alloc_semaphore("out_sem")

    def blk(t, i, n=1):
        return t[:, i * PC:(i + n) * PC]

    with nc.Block() as b:
        def sync_f(sync):
            sync.dma_start(qa_s[:], qa_d[:, :]).then_inc(in_sem, 16)
            sync.dma_start(qb_s[:], qb_d[:, :]).then_inc(in_sem, 16)
            sync.wait_ge(done_sem, 1)
            sync.dma_start(r_d[:, :], rt[:]).then_inc(out_sem, 16)
            sync.wait_ge(out_sem, 16)

        def vec_f(vec):
            vec.wait_ge(in_sem, 32)
            vec.tensor_tensor(out=prod[:], in0=qa_s[:], in1=qb_s[:],
                              op=OP.mult)
            vec.tensor_tensor(out=plus[:], in0=blk(prod, 0, 6),
                              in1=blk(prod, 6, 6), op=OP.add)
            vec.tensor_tensor(out=minus[:], in0=blk(prod, 12, 3),
                              in1=blk(prod, 15, 3), op=OP.subtract)
            vec.tensor_tensor(out=s2[:], in0=blk(prod, 18, 2),
                              in1=blk(prod, 20, 2), op=OP.add)
            vec.tensor_tensor(out=d1[:], in0=blk(s2, 0), in1=blk(s2, 1),
                              op=OP.add)
            vec.tensor_scalar(out=dh[:], in0=d1[:], scalar1=0.5, scalar2=None,
                              op0=OP.mult)
            vec.reciprocal(u[:], dh[:])        # u = 2/|q|^2
            # off-diagonals: R order R00 R01 R02 R10 R11 R12 R20 R21 R22
            for src, dst in ((3, 3), (4, 2), (5, 7)):    # plus -> o10 o02 o21
                vec.tensor_tensor(out=blk(rt, dst), in0=blk(plus, src),
                                  in1=u[:], op=OP.mult)
            for src, dst in ((0, 1), (1, 6), (2, 5)):   # minus -> o01 o20 o12
                vec.tensor_tensor(out=blk(rt, dst), in0=blk(minus, src),
                                  in1=u[:], op=OP.mult)
            # diagonals: R_ii = 1 - u*(pair)
            for i in range(3):
                vec.tensor_tensor(out=blk(dgm, i), in0=blk(plus, i),
                                  in1=u[:], op=OP.mult)
            for i, dst in enumerate((0, 4, 8)):
                ins = vec.tensor_scalar(out=blk(rt, dst), in0=blk(dgm, i),
                                        scalar1=-1.0, scalar2=1.0,
                                        op0=OP.mult, op1=OP.add)
            ins.then_inc(done_sem, 1)

        b.sync(sync_f)
        b.vector(vec_f)
    nc.finalize()
    return nc


def _build_main():
    import concourse.tile as tile
    from concourse import bacc, mybir

    nc = bacc.Bacc("TRN2", target_bir_lowering=False, debug=False,
                   num_devices=NCORES)
    f16 = mybir.dt.float16
    f32 = mybir.dt.float32
    AF = mybir.ActivationFunctionType
    OP = mybir.AluOpType
    tR_d = nc.declare_dram_parameter("tR", [NCH, P, 9 * CC], f16,
                                     isOutput=False)
    tP_d = nc.declare_dram_parameter("tP", [NCH, P, 3 * CC], f16,
                                     isOutput=False)
    tTW_d = nc.declare_dram_parameter("tTW", [NCH, P, 4 * CC], f16,
                                      isOutput=False)
    tXY_d = nc.declare_dram_parameter("tXY", [NCH, P, 3 * CC], f16,
                                      isOutput=False)
    out_d = nc.declare_dram_parameter("out", [NCH, P, 3 * CC], f16,
                                      isOutput=True)

    with tile.TileContext(nc) as tc:
        with tc.tile_pool(name="inp", bufs=2) as inp, \
             tc.tile_pool(name="tp", bufs=2) as tp, \
             nc.allow_low_precision(reason="fp16 pipeline, rel gate 2e-2"):
            vec, act = nc.vector, nc.scalar
            for ch in range(NCH):
                tR = inp.tile([P, 9, CC], f16, tag="tR", name=f"tR{ch}")
                nc.sync.dma_start(tR[:], tR_d[ch])
                tP = inp.tile([P, 3, CC], f16, tag="tP", name=f"tP{ch}")
                nc.sync.dma_start(tP[:], tP_d[ch])
                tTW = inp.tile([P, 4, CC], f16, tag="tTW", name=f"tTW{ch}")
                nc.sync.dma_start(tTW[:], tTW_d[ch])
                tXY = inp.tile([P, 3, CC], f16, tag="tXY", name=f"tXY{ch}")
                nc.sync.dma_start(tXY[:], tXY_d[ch])
                ot = tp.tile([P, 3, CC], f16, tag="out", name=f"out{ch}")

                def TL(shape, dt, tag, bufs):
                    return tp.tile(shape, dt, tag=tag, name=f"{tag}{ch}",
                                   bufs=bufs)

                T3 = tTW[:, 0:3, :]                     # tx ty tz
                Wp = tTW[:, 3, :]                       # weights
                X, Y, Z = tXY[:, 0, :], tXY[:, 1, :], tXY[:, 2, :]

                # all 9 rotation products in one op: R9 * [p3 p3 p3]
                m9 = TL([P, 9, CC], f16, "m9", 1)
                p3b = tP[:].unsqueeze(1).broadcast_to([P, 3, 3, CC])
                vec.tensor_tensor(out=m9[:], in0=tR[:], in1=p3b, op=OP.mult)
                # row sums + translation: r3 = [rx ry rz]
                s1 = TL([P, 3, CC], f16, "s1", 1)
                vec.tensor_tensor(out=s1[:], in0=m9[:, 0::3, :],
                                  in1=m9[:, 1::3, :], op=OP.add)
                s2 = TL([P, 3, CC], f16, "s2", 1)
                vec.tensor_tensor(out=s2[:], in0=s1[:], in1=m9[:, 2::3, :],
                                  op=OP.add)
                r3 = TL([P, 3, CC], f16, "r3", 2)
                vec.tensor_tensor(out=r3[:], in0=s2[:], in1=T3, op=OP.add)
                rx, ry, rz = r3[:, 0, :], r3[:, 1, :], r3[:, 2, :]

                # squares (one op), rho2/r2 packed pair, sqrt pair
                sq3 = TL([P, 3, CC], f16, "sq3", 2)
                act.activation(sq3[:], r3[:], AF.Square)
                rr = TL([P, 2, CC], f16, "rr", 2)
                vec.tensor_tensor(out=rr[:, 0, :], in0=sq3[:, 0, :],
                                  in1=sq3[:, 1, :], op=OP.add)
                vec.tensor_tensor(out=rr[:, 1, :], in0=rr[:, 0, :],
                                  in1=sq3[:, 2, :], op=OP.add)
                sr = TL([P, 2, CC], f16, "sr", 2)
                act.activation(sr[:], rr[:], AF.Sqrt)   # [rho | rng]
                rho, rng = sr[:, 0, :], sr[:, 1, :]

                # fp16 guarded reciprocals; clamps kill any inf before atan
                rho_g = TL([P, CC], f16, "rhog", 1)
                vec.tensor_scalar(out=rho_g[:], in0=rho, scalar1=6e-5,
                                  scalar2=None, op0=OP.max)
                irho = TL([P, CC], f16, "irho", 1)
                vec.reciprocal(irho[:], rho_g[:])
                den = TL([P, CC], f16, "den", 1)
                vec.tensor_tensor(out=den[:], in0=rho_g[:], in1=rx, op=OP.add)
                den_g = TL([P, CC], f16, "deng", 1)
                vec.tensor_scalar(out=den_g[:], in0=den[:], scalar1=6e-5,
                                  scalar2=None, op0=OP.max)
                iden = TL([P, CC], f16, "iden", 1)
                vec.reciprocal(iden[:], den_g[:])
                q = TL([P, CC], f16, "q", 1)
                vec.tensor_tensor(out=q[:], in0=ry, in1=iden[:], op=OP.mult)
                e1 = TL([P, CC], f16, "e1", 1)
                vec.tensor_tensor(out=e1[:], in0=rz, in1=irho[:], op=OP.mult)
                # atan args packed [q | e1], clamped to +-3e4
                qa2 = TL([P, 2, CC], f16, "qa2", 1)
                vec.tensor_scalar(out=qa2[:, 0, :], in0=q[:], scalar1=3e4,
                                  scalar2=-3e4, op0=OP.min, op1=OP.max)
                vec.tensor_scalar(out=qa2[:, 1, :], in0=e1[:], scalar1=3e4,
                                  scalar2=-3e4, op0=OP.min, op1=OP.max)
                at2 = TL([P, 2, CC], f16, "at2", 2)
                act.activation(at2[:], qa2[:], AF.Arctan)  # [az0 | el]
                az2 = TL([P, CC], f16, "az2", 2)
                act.activation(az2[:], at2[:, 0, :], AF.Copy, scale=2.0)

                # residuals: d3 = [rng-X | 2*az0-Y | el-Z], out = d3*W
                d3 = TL([P, 3, CC], f16, "d3", 1)
                vec.tensor_tensor(out=d3[:, 0, :], in0=rng, in1=X,
                                  op=OP.subtract)
                vec.tensor_tensor(out=d3[:, 1, :], in0=az2[:], in1=Y,
                                  op=OP.subtract)
                vec.tensor_tensor(out=d3[:, 2, :], in0=at2[:, 1, :], in1=Z,
                                  op=OP.subtract)
                wb = Wp.unsqueeze(1).broadcast_to([P, 3, CC])
                vec.tensor_tensor(out=ot[:], in0=d3[:], in1=wb, op=OP.mult)
                nc.scalar.dma_start(out_d[ch], ot[:])
    nc.finalize()
    return nc


def _get(name, builder):
    if name not in _CACHE:
        _CACHE[name] = builder()
    return _CACHE[name]


def stage_q(poses):
    """(qa, qb) [128, NQB*32] f32 operand planes for launch A's one big mult."""
    qp = poses[:, 3:7].reshape(P, PC, 4).transpose(2, 0, 1)  # [4,128,32]
    qa = np.concatenate([qp[i] for i in _QA_IDX], axis=1)
    qb = np.concatenate([qp[i] for i in _QB_IDX], axis=1)
    return np.ascontiguousarray(qa), np.ascontiguousarray(qb)


def decode_rtab(raw):
    """[128, 9*32] device layout -> [4096, 9] table."""
    r = np.asarray(raw).reshape(P, 9, PC).transpose(0, 2, 1)  # [128, 32, 9]
    return np.ascontiguousarray(r.reshape(NPOSE, 9))


def stage_obs(rtab, poses, patch_coords, elevation_angle, pid, qid,
              target_coords, weights):
    """Gather per-observation planes, cast fp16, lay out per core/chunk.

    Returns dict of arrays [NCORES, NCH, P, n, CC] f16 keyed like the
    kernel's dram parameters.
    """
    r9 = rtab[pid]                                            # [M, 9]
    t3 = poses[pid, 0:3]                                      # [M, 3]
    pts = np.concatenate(
        [patch_coords[qid], elevation_angle[qid]], axis=1)    # [M, 3]

    def lay(v):
        np_ = v.shape[1]
        v = v.astype(np.float16)
        v = v.reshape(NCORES, P, NCH, CC, np_).transpose(0, 2, 1, 4, 3)
        return np.ascontiguousarray(v)

    return {
        "tR": lay(r9),
        "tP": lay(pts),
        "tTW": lay(np.concatenate([t3, weights], axis=1)),
        "tXY": lay(target_coords),
    }


def unstage_out(res_list):
    """res_list: per-core [NCH,P,3,CC] f16 -> [M,3] f32."""
    out = np.stack([np.asarray(r).reshape(NCH, P, 3, CC) for r in res_list])
    out = out.transpose(0, 2, 1, 4, 3).reshape(M, 3)
    return np.ascontiguousarray(out).astype(np.float32)


def kernel(poses, patch_coords, elevation_angle, poses_idx, patch_idx,
           target_coords, weights):
    from concourse.bass_utils import run_bass_kernel_spmd

    poses = np.asarray(poses, dtype=np.float32)
    patch_coords = np.asarray(patch_coords, dtype=np.float32)
    elevation_angle = np.asarray(elevation_angle, dtype=np.float32)
    target_coords = np.asarray(target_coords, dtype=np.float32)
    weights = np.asarray(weights, dtype=np.float32)
    pid = np.asarray(poses_idx).astype(np.int64)
    qid = np.asarray(patch_idx).astype(np.int64)

    # ---- launch A: pose table -> rotation matrices (device) ----
    qa, qb = stage_q(poses)
    ncA = _get("A", _build_posetab)
    resA = run_bass_kernel_spmd(ncA,
                                [{"qa": qa, "qb": qb} for _ in range(NCORES)],
                                list(range(NCORES)))
    rtab = decode_rtab(resA.results[0]["rtab"])

    # ---- host: gather + fp16 staging (indexing/layout only) ----
    staged = stage_obs(rtab, poses, patch_coords, elevation_angle,
                       pid, qid, target_coords, weights)

    # ---- launch B: streaming rotate+polar+residual ----
    ncB = _get("B", _build_main)
    resB = run_bass_kernel_spmd(
        ncB, [{k: v[c] for k, v in staged.items()} for c in range(NCORES)],
        list(range(NCORES)))
    return unstage_out([resB.results[c]["out"] for c in range(NCORES)])


# revision 28
# speedup vs baseline: 1.2624x; 1.2624x over previous
"""Trainium2 Bass kernel for nn_BoundleAdjustment (2M observations).

Two launches on all 8 NeuronCores (observations data-parallel, M/8 per core):

Launch A (device): converts the 4096-row pose table (translation+quaternion)
into per-pose rotation matrices R = f(q/|q|) on the Vector engine
([128, 32] planar layout, one reciprocal for the 2/|q|^2 scale).

Host staging (indexing/layout only): gathers the derived R table, raw pose
translations, and patch rows by poses_idx/patch_idx, casts the per-
observation record planes to fp16, and lays them out as two contiguous
blocks per chunk so each chunk needs only two big DMAs.

Launch B (device): streams fp16 planes through SBUF in 2 chunks.
Rotation + residual math runs in fp16 on the Vector engine (2x DVE mode);
squares/sqrts/arctans on the Scalar engine; the azimuth uses the
half-angle identity az = 2*atan(ry/(rho+rx)) which needs no quadrant
fixup; the two reciprocals run in f32 via reciprocal_approx_fast with
max(x,1e-30) guards so no inf/NaN can form.
"""

import numpy as np

M = 2097152
NCORES = 8
N = M // NCORES
P = 128
COLS = N // P            # 2048
CHUNKS = [256, 896, 896]  # small first chunk -> compute starts early
NPOSE = 4096
PC = NPOSE // P          # 32 cols for pose table

# input groups (fp16), DMA'd per chunk in need-order so compute starts
# as soon as the rotation operands have landed:
#   tR:  R00 R01 R02 R10 R11 R12 R20 R21 R22   (9 planes, first)
#   tP:  px py pz                              (3 planes)
#   tTW: tx ty tz W                            (4 planes)
#   tXY: X Y Z                                 (3 planes, last)
NPA = 7                  # legacy (unused)
NPB_ = 12                # legacy (unused)

_CACHE = {}


# launch A staged layout: 22 blocks of 32 cols, products prod_k = QA_k * QB_k
#   0-5   PL1 = yy xx xx xy xz yz      6-11  PL2 = zz zz yy wz wy wx
#   12-14 MN1 = xy xz yz               15-17 MN2 = wz wy wx
#   18-21 SS  = xx yy zz ww
# plus  = PL1+PL2 = [d00 d11 d22 o10 o02 o21], minus = MN1-MN2 = [o01 o20 o12]
_QA_IDX = [1, 0, 0, 0, 0, 1,  2, 2, 1, 3, 3, 3,  0, 0, 1,  3, 3, 3,  0, 1, 2, 3]
_QB_IDX = [1, 0, 0, 1, 2, 2,  2, 2, 1, 2, 1, 0,  1, 2, 2,  2, 1, 0,  0, 1, 2, 3]
NQB = 22


def _build_posetab():
    import concourse.tile as tile
    from concourse import bacc, mybir

    nc = bacc.Bacc("TRN2", target_bir_lowering=False, debug=False,
                   num_devices=NCORES)
    f32 = mybir.dt.float32
    OP = mybir.AluOpType
    qa_d = nc.declare_dram_parameter("qa", [P, NQB * PC], f32, isOutput=False)
    qb_d = nc.declare_dram_parameter("qb", [P, NQB * PC], f32, isOutput=False)
    r_d = nc.declare_dram_parameter("rtab", [P, 9 * PC], f32, isOutput=True)

    with tile.TileContext(nc) as tc:
        with tc.tile_pool(name="pp", bufs=12) as pp:
            vec = nc.vector
            qa = pp.tile([P, NQB * PC], f32, tag="qa", name="qa")
            nc.sync.dma_start(qa[:], qa_d[:, :])
            qb = pp.tile([P, NQB * PC], f32, tag="qb", name="qb")
            nc.sync.dma_start(qb[:], qb_d[:, :])
            rt = pp.tile([P, 9 * PC], f32, tag="rt", name="rt")

            def blk(t, i, n=1):
                return t[:, i * PC:(i + n) * PC]

            prod = pp.tile([P, NQB * PC], f32, tag="prod", name="prod")
            vec.tensor_tensor(out=prod[:], in0=qa[:], in1=qb[:], op=OP.mult)
            plus = pp.tile([P, 6 * PC], f32, tag="plus", name="plus")
            vec.tensor_tensor(out=plus[:], in0=blk(prod, 0, 6),
                              in1=blk(prod, 6, 6), op=OP.add)
            minus = pp.tile([P, 3 * PC], f32, tag="minus", name="minus")
            vec.tensor_tensor(out=minus[:], in0=blk(prod, 12, 3),
                              in1=blk(prod, 15, 3), op=OP.subtract)
            s2 = pp.tile([P, 2 * PC], f32, tag="s2", name="s2")
            vec.tensor_tensor(out=s2[:], in0=blk(prod, 18, 2),
                              in1=blk(prod, 20, 2), op=OP.add)
            d1 = pp.tile([P, PC], f32, tag="d1", name="d1")
            vec.tensor_tensor(out=d1[:], in0=blk(s2, 0), in1=blk(s2, 1),
                              op=OP.add)
            dh = pp.tile([P, PC], f32, tag="dh", name="dh")
            vec.tensor_scalar(out=dh[:], in0=d1[:], scalar1=0.5, scalar2=None,
                              op0=OP.mult)
            u = pp.tile([P, PC], f32, tag="u", name="u")
            vec.reciprocal(u[:], dh[:])        # u = 2/|q|^2

            # off-diagonals: R order R00 R01 R02 R10 R11 R12 R20 R21 R22
            for src, dst in ((3, 3), (4, 2), (5, 7)):    # plus -> o10 o02 o21
                vec.tensor_tensor(out=blk(rt, dst), in0=blk(plus, src),
                                  in1=u[:], op=OP.mult)
            for src, dst in ((0, 1), (1, 6), (2, 5)):   # minus -> o01 o20 o12
                vec.tensor_tensor(out=blk(rt, dst), in0=blk(minus, src),
                                  in1=u[:], op=OP.mult)
            # diagonals: R_ii = 1 - u*(pair)
            dgm = pp.tile([P, 3 * PC], f32, tag="dgm", name="dgm")
            for i in range(3):
                vec.tensor_tensor(out=blk(dgm, i), in0=blk(plus, i),
                                  in1=u[:], op=OP.mult)
            for i, dst in enumerate((0, 4, 8)):
                vec.tensor_scalar(out=blk(rt, dst), in0=blk(dgm, i),
                                  scalar1=-1.0, scalar2=1.0,
                                  op0=OP.mult, op1=OP.add)
            nc.sync.dma_start(r_d[:, :], rt[:])
    nc.finalize()
    return nc


def _build_main():
    import concourse.tile as tile
    from concourse import bacc, mybir

    nc = bacc.Bacc("TRN2", target_bir_lowering=False, debug=False,
                   num_devices=NCORES)
    f16 = mybir.dt.float16
    f32 = mybir.dt.float32
    AF = mybir.ActivationFunctionType
    OP = mybir.AluOpType
    tR_d = nc.declare_dram_parameter("tR", [P, 9 * COLS], f16,
                                     isOutput=False)
    tP_d = nc.declare_dram_parameter("tP", [P, 3 * COLS], f16,
                                     isOutput=False)
    tTW_d = nc.declare_dram_parameter("tTW", [P, 4 * COLS], f16,
                                      isOutput=False)
    tXY_d = nc.declare_dram_parameter("tXY", [P, 3 * COLS], f16,
                                      isOutput=False)
    out_d = nc.declare_dram_parameter("out", [P, 3 * COLS], f16,
                                      isOutput=True)

    with tile.TileContext(nc) as tc:
        with tc.tile_pool(name="inp", bufs=2) as inp, \
             tc.tile_pool(name="tp", bufs=2) as tp:
            vec, act = nc.vector, nc.scalar
            off = 0
            for ch, CC in enumerate(CHUNKS):
                o9, o3, o4 = 9 * off, 3 * off, 4 * off
                c9, c3, c4 = 9 * CC, 3 * CC, 4 * CC
                tR = inp.tile([P, 9, CC], f16, tag="tR", name=f"tR{ch}")
                nc.sync.dma_start(tR[:], tR_d[:, o9:o9 + c9])
                tP = inp.tile([P, 3, CC], f16, tag="tP", name=f"tP{ch}")
                nc.sync.dma_start(tP[:], tP_d[:, o3:o3 + c3])
                tTW = inp.tile([P, 4, CC], f16, tag="tTW", name=f"tTW{ch}")
                nc.sync.dma_start(tTW[:], tTW_d[:, o4:o4 + c4])
                tXY = inp.tile([P, 3, CC], f16, tag="tXY", name=f"tXY{ch}")
                nc.sync.dma_start(tXY[:], tXY_d[:, o3:o3 + c3])
                ot = tp.tile([P, 3, CC], f16, tag="out", name=f"out{ch}")

                def TL(shape, dt, tag, bufs):
                    return tp.tile(shape, dt, tag=tag, name=f"{tag}{ch}",
                                   bufs=bufs)

                T3 = tTW[:, 0:3, :]                     # tx ty tz
                Wp = tTW[:, 3, :]                       # weights
                X, Y, Z = tXY[:, 0, :], tXY[:, 1, :], tXY[:, 2, :]

                # all 9 rotation products in one op: R9 * [p3 p3 p3]
                m9 = TL([P, 9, CC], f16, "m9", 2)
                p3b = tP[:].unsqueeze(1).broadcast_to([P, 3, 3, CC])
                vec.tensor_tensor(out=m9[:], in0=tR[:], in1=p3b, op=OP.mult)
                # row sums + translation: r3 = [rx ry rz]
                s1 = TL([P, 3, CC], f16, "s1", 1)
                vec.tensor_tensor(out=s1[:], in0=m9[:, 0::3, :],
                                  in1=m9[:, 1::3, :], op=OP.add)
                s2 = TL([P, 3, CC], f16, "s2", 1)
                vec.tensor_tensor(out=s2[:], in0=s1[:], in1=m9[:, 2::3, :],
                                  op=OP.add)
                r3 = TL([P, 3, CC], f16, "r3", 2)
                vec.tensor_tensor(out=r3[:], in0=s2[:], in1=T3, op=OP.add)
                rx, ry, rz = r3[:, 0, :], r3[:, 1, :], r3[:, 2, :]

                # squares (one op), rho2/r2 packed pair, sqrt pair
                sq3 = TL([P, 3, CC], f16, "sq3", 2)
                act.activation(sq3[:], r3[:], AF.Square)
                rr = TL([P, 2, CC], f16, "rr", 2)
                vec.tensor_tensor(out=rr[:, 0, :], in0=sq3[:, 0, :],
                                  in1=sq3[:, 1, :], op=OP.add)
                vec.tensor_tensor(out=rr[:, 1, :], in0=rr[:, 0, :],
                                  in1=sq3[:, 2, :], op=OP.add)
                sr = TL([P, 2, CC], f16, "sr", 2)
                act.activation(sr[:], rr[:], AF.Sqrt)   # [rho | rng]
                rho, rng = sr[:, 0, :], sr[:, 1, :]

                # guarded reciprocals in f32 (no inf/NaN possible)
                rho_g = TL([P, CC], f32, "rhog", 1)
                vec.tensor_scalar(out=rho_g[:], in0=rho, scalar1=1e-30,
                                  scalar2=None, op0=OP.max)
                irho = TL([P, CC], f32, "irho", 1)
                vec.reciprocal_approx_fast(irho[:], rho_g[:])
                den = TL([P, CC], f32, "den", 1)
                vec.tensor_tensor(out=den[:], in0=rho_g[:], in1=rx, op=OP.add)
                den_g = TL([P, CC], f32, "deng", 1)
                vec.tensor_scalar(out=den_g[:], in0=den[:], scalar1=1e-30,
                                  scalar2=None, op0=OP.max)
                iden = TL([P, CC], f32, "iden", 1)
                vec.reciprocal_approx_fast(iden[:], den_g[:])
                # atan args packed: [ry/(rho+rx) | rz/rho]
                qa2 = TL([P, 2, CC], f32, "qa2", 1)
                vec.tensor_tensor(out=qa2[:, 0, :], in0=ry, in1=iden[:],
                                  op=OP.mult)
                vec.tensor_tensor(out=qa2[:, 1, :], in0=rz, in1=irho[:],
                                  op=OP.mult)
                at2 = TL([P, 2, CC], f16, "at2", 2)
                act.activation(at2[:], qa2[:], AF.Arctan)  # [az0 | el]
                az2 = TL([P, CC], f16, "az2", 2)
                act.activation(az2[:], at2[:, 0, :], AF.Copy, scale=2.0)

                # residuals: d3 = [rng-X | 2*az0-Y | el-Z], out = d3*W
                d3 = TL([P, 3, CC], f16, "d3", 1)
                vec.tensor_tensor(out=d3[:, 0, :], in0=rng, in1=X,
                                  op=OP.subtract)
                vec.tensor_tensor(out=d3[:, 1, :], in0=az2[:], in1=Y,
                                  op=OP.subtract)
                vec.tensor_tensor(out=d3[:, 2, :], in0=at2[:, 1, :], in1=Z,
                                  op=OP.subtract)
                wb = Wp.unsqueeze(1).broadcast_to([P, 3, CC])
                vec.tensor_tensor(out=ot[:], in0=d3[:], in1=wb, op=OP.mult)
                nc.scalar.dma_start(out_d[:, o3:o3 + c3], ot[:])
                off += CC
    nc.finalize()
    return nc


def _get(name, builder):
    if name not in _CACHE:
        _CACHE[name] = builder()
    return _CACHE[name]


def stage_q(poses):
    """(qa, qb) [128, NQB*32] f32 operand planes for launch A's one big mult."""
    qp = poses[:, 3:7].reshape(P, PC, 4).transpose(2, 0, 1)  # [4,128,32]
    qa = np.concatenate([qp[i] for i in _QA_IDX], axis=1)
    qb = np.concatenate([qp[i] for i in _QB_IDX], axis=1)
    return np.ascontiguousarray(qa), np.ascontiguousarray(qb)


def decode_rtab(raw):
    """[128, 9*32] device layout -> [4096, 9] table."""
    r = np.asarray(raw).reshape(P, 9, PC).transpose(0, 2, 1)  # [128, 32, 9]
    return np.ascontiguousarray(r.reshape(NPOSE, 9))


def stage_obs(rtab, poses, patch_coords, elevation_angle, pid, qid,
              target_coords, weights):
    """Gather per-observation planes, cast fp16, lay out per core/chunk.

    Returns dict of arrays [NCORES, P, n*COLS] f16 keyed like the kernel's
    dram parameters; each chunk's planes are chunk-major contiguous.
    """
    r9 = rtab[pid]                                            # [M, 9]
    t3 = poses[pid, 0:3]                                      # [M, 3]
    pts = np.concatenate(
        [patch_coords[qid], elevation_angle[qid]], axis=1)    # [M, 3]

    def lay(v):
        np_ = v.shape[1]
        v = v.astype(np.float16).reshape(NCORES, P, COLS, np_)
        parts, off = [], 0
        for cc in CHUNKS:
            blk = v[:, :, off:off + cc, :].transpose(0, 1, 3, 2)
            parts.append(blk.reshape(NCORES, P, np_ * cc))
            off += cc
        return np.ascontiguousarray(np.concatenate(parts, axis=2))

    return {
        "tR": lay(r9),
        "tP": lay(pts),
        "tTW": lay(np.concatenate([t3, weights], axis=1)),
        "tXY": lay(target_coords),
    }


def unstage_out(res_list):
    """res_list: per-core [P, 3*COLS] f16 (chunk-major) -> [M,3] f32."""
    full = np.empty((NCORES, P, COLS, 3), dtype=np.float32)
    for c, r in enumerate(res_list):
        r = np.asarray(r).reshape(P, 3 * COLS)
        off = 0
        for cc in CHUNKS:
            blk = r[:, 3 * off:3 * (off + cc)].reshape(P, 3, cc)
            full[c, :, off:off + cc, :] = blk.transpose(0, 2, 1)
            off += cc
    return np.ascontiguousarray(full.reshape(M, 3))


def kernel(poses, patch_coords, elevation_angle, poses_idx, patch_idx,
           target_coords, weights):
    from concourse.bass_utils import run_bass_kernel_spmd

    poses = np.asarray(poses, dtype=np.float32)
    patch_coords = np.asarray(patch_coords, dtype=np.float32)
    elevation_angle = np.asarray(elevation_angle, dtype=np.float32)
    target_coords = np.asarray(target_coords, dtype=np.float32)
    weights = np.asarray(weights, dtype=np.float32)
    pid = np.asarray(poses_idx).astype(np.int64)
    qid = np.asarray(patch_idx).astype(np.int64)

    # ---- launch A: pose table -> rotation matrices (device) ----
    qa, qb = stage_q(poses)
    ncA = _get("A", _build_posetab)
    resA = run_bass_kernel_spmd(ncA,
                                [{"qa": qa, "qb": qb} for _ in range(NCORES)],
                                list(range(NCORES)))
    rtab = decode_rtab(resA.results[0]["rtab"])

    # ---- host: gather + fp16 staging (indexing/layout only) ----
    staged = stage_obs(rtab, poses, patch_coords, elevation_angle,
                       pid, qid, target_coords, weights)

    # ---- launch B: streaming rotate+polar+residual ----
    ncB = _get("B", _build_main)
    resB = run_bass_kernel_spmd(
        ncB, [{k: v[c] for k, v in staged.items()} for c in range(NCORES)],
        list(range(NCORES)))
    return unstage_out([resB.results[c]["out"] for c in range(NCORES)])


# revision 30
# speedup vs baseline: 1.2878x; 1.0201x over previous
"""Trainium2 Bass kernel for nn_BoundleAdjustment (2M observations).

Two launches on all 8 NeuronCores (observations data-parallel, M/8 per core):

Launch A (device): converts the 4096-row pose table (translation+quaternion)
into per-pose rotation matrices R = f(q/|q|) on the Vector engine
([128, 32] planar layout, one reciprocal for the 2/|q|^2 scale).

Host staging (indexing/layout only): gathers the derived R table, raw pose
translations, and patch rows by poses_idx/patch_idx, casts the per-
observation record planes to fp16, and lays them out as two contiguous
blocks per chunk so each chunk needs only two big DMAs.

Launch B (device): streams fp16 planes through SBUF in 2 chunks.
Rotation + residual math runs in fp16 on the Vector engine (2x DVE mode);
squares/sqrts/arctans on the Scalar engine; the azimuth uses the
half-angle identity az = 2*atan(ry/(rho+rx)) which needs no quadrant
fixup; the two reciprocals run in f32 via reciprocal_approx_fast with
max(x,1e-30) guards so no inf/NaN can form.
"""

import numpy as np

M = 2097152
NCORES = 8
N = M // NCORES
P = 128
COLS = N // P            # 2048
CHUNKS = [256, 896, 896]  # small first chunk -> compute starts early
NPOSE = 4096
PC = NPOSE // P          # 32 cols for pose table

# input groups (fp16), DMA'd per chunk in need-order so compute starts
# as soon as the rotation operands have landed:
#   tR:  R00 R01 R02 R10 R11 R12 R20 R21 R22   (9 planes, first)
#   tP:  px py pz                              (3 planes)
#   tTW: tx ty tz W                            (4 planes)
#   tXY: X Y Z                                 (3 planes, last)
NPA = 7                  # legacy (unused)
NPB_ = 12                # legacy (unused)

_CACHE = {}


# launch A staged layout: 22 blocks of 32 cols, products prod_k = QA_k * QB_k
#   0-5   PL1 = yy xx xx xy xz yz      6-11  PL2 = zz zz yy wz wy wx
#   12-14 MN1 = xy xz yz               15-17 MN2 = wz wy wx
#   18-21 SS  = xx yy zz ww
# plus  = PL1+PL2 = [d00 d11 d22 o10 o02 o21], minus = MN1-MN2 = [o01 o20 o12]
_QA_IDX = [1, 0, 0, 0, 0, 1,  2, 2, 1, 3, 3, 3,  0, 0, 1,  3, 3, 3,  0, 1, 2, 3]
_QB_IDX = [1, 0, 0, 1, 2, 2,  2, 2, 1, 2, 1, 0,  1, 2, 2,  2, 1, 0,  0, 1, 2, 3]
NQB = 22


def _build_posetab():
    import concourse.tile as tile
    from concourse import bacc, mybir

    nc = bacc.Bacc("TRN2", target_bir_lowering=False, debug=False,
                   num_devices=NCORES)
    f32 = mybir.dt.float32
    OP = mybir.AluOpType
    qa_d = nc.declare_dram_parameter("qa", [P, NQB * PC], f32, isOutput=False)
    qb_d = nc.declare_dram_parameter("qb", [P, NQB * PC], f32, isOutput=False)
    r_d = nc.declare_dram_parameter("rtab", [P, 9 * PC], f32, isOutput=True)

    with tile.TileContext(nc) as tc:
        with tc.tile_pool(name="pp", bufs=12) as pp:
            vec = nc.vector
            qa = pp.tile([P, NQB * PC], f32, tag="qa", name="qa")
            nc.sync.dma_start(qa[:], qa_d[:, :])
            qb = pp.tile([P, NQB * PC], f32, tag="qb", name="qb")
            nc.sync.dma_start(qb[:], qb_d[:, :])
            rt = pp.tile([P, 9 * PC], f32, tag="rt", name="rt")

            def blk(t, i, n=1):
                return t[:, i * PC:(i + n) * PC]

            prod = pp.tile([P, NQB * PC], f32, tag="prod", name="prod")
            vec.tensor_tensor(out=prod[:], in0=qa[:], in1=qb[:], op=OP.mult)
            plus = pp.tile([P, 6 * PC], f32, tag="plus", name="plus")
            vec.tensor_tensor(out=plus[:], in0=blk(prod, 0, 6),
                              in1=blk(prod, 6, 6), op=OP.add)
            minus = pp.tile([P, 3 * PC], f32, tag="minus", name="minus")
            vec.tensor_tensor(out=minus[:], in0=blk(prod, 12, 3),
                              in1=blk(prod, 15, 3), op=OP.subtract)
            s2 = pp.tile([P, 2 * PC], f32, tag="s2", name="s2")
            vec.tensor_tensor(out=s2[:], in0=blk(prod, 18, 2),
                              in1=blk(prod, 20, 2), op=OP.add)
            d1 = pp.tile([P, PC], f32, tag="d1", name="d1")
            vec.tensor_tensor(out=d1[:], in0=blk(s2, 0), in1=blk(s2, 1),
                              op=OP.add)
            dh = pp.tile([P, PC], f32, tag="dh", name="dh")
            vec.tensor_scalar(out=dh[:], in0=d1[:], scalar1=0.5, scalar2=None,
                              op0=OP.mult)
            u = pp.tile([P, PC], f32, tag="u", name="u")
            vec.reciprocal(u[:], dh[:])        # u = 2/|q|^2

            # off-diagonals: R order R00 R01 R02 R10 R11 R12 R20 R21 R22
            for src, dst in ((3, 3), (4, 2), (5, 7)):    # plus -> o10 o02 o21
                vec.tensor_tensor(out=blk(rt, dst), in0=blk(plus, src),
                                  in1=u[:], op=OP.mult)
            for src, dst in ((0, 1), (1, 6), (2, 5)):   # minus -> o01 o20 o12
                vec.tensor_tensor(out=blk(rt, dst), in0=blk(minus, src),
                                  in1=u[:], op=OP.mult)
            # diagonals: R_ii = 1 - u*(pair)
            dgm = pp.tile([P, 3 * PC], f32, tag="dgm", name="dgm")
            for i in range(3):
                vec.tensor_tensor(out=blk(dgm, i), in0=blk(plus, i),
                                  in1=u[:], op=OP.mult)
            for i, dst in enumerate((0, 4, 8)):
                vec.tensor_scalar(out=blk(rt, dst), in0=blk(dgm, i),
                                  scalar1=-1.0, scalar2=1.0,
                                  op0=OP.mult, op1=OP.add)
            nc.sync.dma_start(r_d[:, :], rt[:])
    nc.finalize()
    return nc


def _build_main():
    import concourse.tile as tile
    from concourse import bacc, mybir

    nc = bacc.Bacc("TRN2", target_bir_lowering=False, debug=False,
                   num_devices=NCORES)
    f16 = mybir.dt.float16
    f32 = mybir.dt.float32
    AF = mybir.ActivationFunctionType
    OP = mybir.AluOpType
    tR_d = nc.declare_dram_parameter("tR", [P, 9 * COLS], f16,
                                     isOutput=False)
    tP_d = nc.declare_dram_parameter("tP", [P, 3 * COLS], f16,
                                     isOutput=False)
    tTW_d = nc.declare_dram_parameter("tTW", [P, 4 * COLS], f16,
                                      isOutput=False)
    tXY_d = nc.declare_dram_parameter("tXY", [P, 3 * COLS], f16,
                                      isOutput=False)
    out_d = nc.declare_dram_parameter("out", [P, 3 * COLS], f16,
                                      isOutput=True)

    with tile.TileContext(nc) as tc:
        with tc.tile_pool(name="inp", bufs=2) as inp, \
             tc.tile_pool(name="tp", bufs=2) as tp:
            vec, act = nc.vector, nc.scalar
            off = 0
            for ch, CC in enumerate(CHUNKS):
                o9, o3, o4 = 9 * off, 3 * off, 4 * off
                c9, c3, c4 = 9 * CC, 3 * CC, 4 * CC
                tR = inp.tile([P, 9, CC], f16, tag="tR", name=f"tR{ch}")
                nc.sync.dma_start(tR[:], tR_d[:, o9:o9 + c9])
                tP = inp.tile([P, 3, CC], f16, tag="tP", name=f"tP{ch}")
                nc.sync.dma_start(tP[:], tP_d[:, o3:o3 + c3])
                tTW = inp.tile([P, 4, CC], f16, tag="tTW", name=f"tTW{ch}")
                nc.sync.dma_start(tTW[:], tTW_d[:, o4:o4 + c4])
                tXY = inp.tile([P, 3, CC], f16, tag="tXY", name=f"tXY{ch}")
                nc.sync.dma_start(tXY[:], tXY_d[:, o3:o3 + c3])
                ot = tp.tile([P, 3, CC], f16, tag="out", name=f"out{ch}")

                def TL(shape, dt, tag, bufs):
                    return tp.tile(shape, dt, tag=tag, name=f"{tag}{ch}",
                                   bufs=bufs)

                T3 = tTW[:, 0:3, :]                     # tx ty tz
                Wp = tTW[:, 3, :]                       # weights
                X, Y, Z = tXY[:, 0, :], tXY[:, 1, :], tXY[:, 2, :]

                # all 9 rotation products in one op: R9 * [p3 p3 p3]
                m9 = TL([P, 9, CC], f16, "m9", 2)
                p3b = tP[:].unsqueeze(1).broadcast_to([P, 3, 3, CC])
                vec.tensor_tensor(out=m9[:], in0=tR[:], in1=p3b, op=OP.mult)
                # row sums + translation: r3 = [rx ry rz]
                s1 = TL([P, 3, CC], f16, "s1", 1)
                vec.tensor_tensor(out=s1[:], in0=m9[:, 0::3, :],
                                  in1=m9[:, 1::3, :], op=OP.add)
                s2 = TL([P, 3, CC], f16, "s2", 1)
                vec.tensor_tensor(out=s2[:], in0=s1[:], in1=m9[:, 2::3, :],
                                  op=OP.add)
                r3 = TL([P, 3, CC], f16, "r3", 2)
                vec.tensor_tensor(out=r3[:], in0=s2[:], in1=T3, op=OP.add)
                rx, ry, rz = r3[:, 0, :], r3[:, 1, :], r3[:, 2, :]

                # squares on DVE (avoids the Square table-load round trip)
                sq3 = TL([P, 3, CC], f16, "sq3", 2)
                vec.tensor_tensor(out=sq3[:], in0=r3[:], in1=r3[:],
                                  op=OP.mult)
                rr = TL([P, 2, CC], f16, "rr", 2)
                vec.tensor_tensor(out=rr[:, 0, :], in0=sq3[:, 0, :],
                                  in1=sq3[:, 1, :], op=OP.add)
                vec.tensor_tensor(out=rr[:, 1, :], in0=rr[:, 0, :],
                                  in1=sq3[:, 2, :], op=OP.add)
                sr = TL([P, 2, CC], f16, "sr", 2)
                act.activation(sr[:], rr[:], AF.Sqrt)   # [rho | rng]
                rho, rng = sr[:, 0, :], sr[:, 1, :]

                # guarded reciprocals in f32 (no inf/NaN possible)
                rho_g = TL([P, CC], f32, "rhog", 1)
                vec.tensor_scalar(out=rho_g[:], in0=rho, scalar1=1e-30,
                                  scalar2=None, op0=OP.max)
                irho = TL([P, CC], f32, "irho", 1)
                vec.reciprocal_approx_fast(irho[:], rho_g[:])
                den = TL([P, CC], f32, "den", 1)
                vec.tensor_tensor(out=den[:], in0=rho_g[:], in1=rx, op=OP.add)
                den_g = TL([P, CC], f32, "deng", 1)
                vec.tensor_scalar(out=den_g[:], in0=den[:], scalar1=1e-30,
                                  scalar2=None, op0=OP.max)
                iden = TL([P, CC], f32, "iden", 1)
                vec.reciprocal_approx_fast(iden[:], den_g[:])
                # atan args packed: [ry/(rho+rx) | rz/rho]
                qa2 = TL([P, 2, CC], f32, "qa2", 1)
                vec.tensor_tensor(out=qa2[:, 0, :], in0=ry, in1=iden[:],
                                  op=OP.mult)
                vec.tensor_tensor(out=qa2[:, 1, :], in0=rz, in1=irho[:],
                                  op=OP.mult)
                at2 = TL([P, 2, CC], f16, "at2", 2)
                act.activation(at2[:], qa2[:], AF.Arctan)  # [az0 | el]
                az2 = TL([P, CC], f16, "az2", 2)
                vec.tensor_scalar(out=az2[:], in0=at2[:, 0, :], scalar1=2.0,
                                  scalar2=None, op0=OP.mult)

                # residuals: d3 = [rng-X | 2*az0-Y | el-Z], out = d3*W
                d3 = TL([P, 3, CC], f16, "d3", 1)
                vec.tensor_tensor(out=d3[:, 0, :], in0=rng, in1=X,
                                  op=OP.subtract)
                vec.tensor_tensor(out=d3[:, 1, :], in0=az2[:], in1=Y,
                                  op=OP.subtract)
                vec.tensor_tensor(out=d3[:, 2, :], in0=at2[:, 1, :], in1=Z,
                                  op=OP.subtract)
                wb = Wp.unsqueeze(1).broadcast_to([P, 3, CC])
                vec.tensor_tensor(out=ot[:], in0=d3[:], in1=wb, op=OP.mult)
                nc.scalar.dma_start(out_d[:, o3:o3 + c3], ot[:])
                off += CC
    nc.finalize()
    return nc


def _get(name, builder):
    if name not in _CACHE:
        _CACHE[name] = builder()
    return _CACHE[name]


def stage_q(poses):
    """(qa, qb) [128, NQB*32] f32 operand planes for launch A's one big mult."""
    qp = poses[:, 3:7].reshape(P, PC, 4).transpose(2, 0, 1)  # [4,128,32]
    qa = np.concatenate([qp[i] for i in _QA_IDX], axis=1)
    qb = np.concatenate([qp[i] for i in _QB_IDX], axis=1)
    return np.ascontiguousarray(qa), np.ascontiguousarray(qb)


def decode_rtab(raw):
    """[128, 9*32] device layout -> [4096, 9] table."""
    r = np.asarray(raw).reshape(P, 9, PC).transpose(0, 2, 1)  # [128, 32, 9]
    return np.ascontiguousarray(r.reshape(NPOSE, 9))


def stage_obs(rtab, poses, patch_coords, elevation_angle, pid, qid,
              target_coords, weights):
    """Gather per-observation planes, cast fp16, lay out per core/chunk.

    Returns dict of arrays [NCORES, P, n*COLS] f16 keyed like the kernel's
    dram parameters; each chunk's planes are chunk-major contiguous.
    """
    r9 = rtab[pid]                                            # [M, 9]
    t3 = poses[pid, 0:3]                                      # [M, 3]
    pts = np.concatenate(
        [patch_coords[qid], elevation_angle[qid]], axis=1)    # [M, 3]

    def lay(v):
        np_ = v.shape[1]
        v = v.astype(np.float16).reshape(NCORES, P, COLS, np_)
        parts, off = [], 0
        for cc in CHUNKS:
            blk = v[:, :, off:off + cc, :].transpose(0, 1, 3, 2)
            parts.append(blk.reshape(NCORES, P, np_ * cc))
            off += cc
        return np.ascontiguousarray(np.concatenate(parts, axis=2))

    return {
        "tR": lay(r9),
        "tP": lay(pts),
        "tTW": lay(np.concatenate([t3, weights], axis=1)),
        "tXY": lay(target_coords),
    }


def unstage_out(res_list):
    """res_list: per-core [P, 3*COLS] f16 (chunk-major) -> [M,3] f32."""
    full = np.empty((NCORES, P, COLS, 3), dtype=np.float32)
    for c, r in enumerate(res_list):
        r = np.asarray(r).reshape(P, 3 * COLS)
        off = 0
        for cc in CHUNKS:
            blk = r[:, 3 * off:3 * (off + cc)].reshape(P, 3, cc)
            full[c, :, off:off + cc, :] = blk.transpose(0, 2, 1)
            off += cc
    return np.ascontiguousarray(full.reshape(M, 3))


def kernel(poses, patch_coords, elevation_angle, poses_idx, patch_idx,
           target_coords, weights):
    from concourse.bass_utils import run_bass_kernel_spmd

    poses = np.asarray(poses, dtype=np.float32)
    patch_coords = np.asarray(patch_coords, dtype=np.float32)
    elevation_angle = np.asarray(elevation_angle, dtype=np.float32)
    target_coords = np.asarray(target_coords, dtype=np.float32)
    weights = np.asarray(weights, dtype=np.float32)
    pid = np.asarray(poses_idx).astype(np.int64)
    qid = np.asarray(patch_idx).astype(np.int64)

    # ---- launch A: pose table -> rotation matrices (device) ----
    qa, qb = stage_q(poses)
    ncA = _get("A", _build_posetab)
    resA = run_bass_kernel_spmd(ncA,
                                [{"qa": qa, "qb": qb} for _ in range(NCORES)],
                                list(range(NCORES)))
    rtab = decode_rtab(resA.results[0]["rtab"])

    # ---- host: gather + fp16 staging (indexing/layout only) ----
    staged = stage_obs(rtab, poses, patch_coords, elevation_angle,
                       pid, qid, target_coords, weights)

    # ---- launch B: streaming rotate+polar+residual ----
    ncB = _get("B", _build_main)
    resB = run_bass_kernel_spmd(
        ncB, [{k: v[c] for k, v in staged.items()} for c in range(NCORES)],
        list(range(NCORES)))
    return unstage_out([resB.results[c]["out"] for c in range(NCORES)])
